# revision 7
# baseline (speedup 1.0000x reference)
"""ModAFNO2D layer as a Bass/Tile kernel for 8 Trainium2 NeuronCores.

Sharding: 8 cores = (batch b in 0..3) x (block-half in 0..1). Each core owns one
batch sample and 4 of the 8 FNO blocks (= 384 of 768 channels). Cores are fully
independent - no collectives; host slices inputs and concatenates outputs.

All matmuls bf16. Host folds the adaLN modulation into the layer-1 weights
(column scaling + bias vectors) and rewrites layer 2 with composite weights so
both its outputs are direct functions of (r1, i1); layer 2 runs data-stationary
(spectrum slice as the PE stationary), landing its output already transposed to
[wf, c] - no explicit PE transposes anywhere.

Stage A exploits Hermitian symmetry of the H-FFT of the real input (h' 0..64
only). Stage B exploits the mirror symmetry spec(128-h') = Zr@cB1 - Zi@cB2:
each mirror pair (h', 128-h') shares the two products T1 = Zr@cB1, T2 = Zi@cB2
(2 matmuls instead of 4); gpsimd forms T1+T2 and T1-T2. Mirrored rows are
stored at permuted positions sigma(p) = p (p<=64) / 192-p (p>64); the inverse-H
DFT matrix is row-permuted on the host to compensate.

Per-core pipeline per block (96 channels):
  A : Zbuf[w, c, (h'r|h'i)] = X_c^T @ [Fr|Fi][:, 0:65]     (FFT along H)
  B : arch[c, hc, (wfr|wfi)] per h'-pair                    (rFFT along W)
  l1: weight-stationary block matmul, modulation pre-folded, relu
  l2: data-stationary -> psum [wf, hc, (r|i)]; softshrink on evict
  E': Pbuf[h', c, (Pr|Pi)] = W_c @ [Sr|Si]                  (inverse rFFT W)
  D': out[h, c, w] = FHrP@Pr + FHiP@Pi + x                  (inverse FFT H)
"""

import numpy as np
import ml_dtypes

BF16 = ml_dtypes.bfloat16

DIM = 768
NB = 8
BS = 96
LAM = 0.01
B_FULL = 4
H = 128
W = 128
WF = W // 2 + 1  # 65
HF = H // 2 + 1  # 65 (Hermitian-reduced H freqs)
NBL = 4          # blocks per core
C = NBL * BS     # 384 channels per core
N_CORES = 8
HC = 4           # h' rows per fused B/mix chunk

# storage permutation: position p holds h' = SIGMA[p]
SIGMA = np.array([p if p <= 64 else 192 - p for p in range(H)])


def _host_consts():
    jh = np.arange(H)
    F = np.exp(-2j * np.pi * np.outer(jh, jh) / H)
    R = np.exp(-2j * np.pi * np.outer(np.arange(WF), np.arange(W)) / W) / 128.0
    RrT, RiT = np.ascontiguousarray(R.real.T), np.ascontiguousarray(R.imag.T)
    FH = np.conj(F)
    cw = np.ones(WF)
    cw[1:-1] = 2.0
    S = (cw[:, None] * np.exp(2j * np.pi * np.outer(np.arange(WF), np.arange(W)) / W)) / 128.0
    cB2 = np.concatenate([-RiT, RrT], 1)
    cDr = FH.real[SIGMA, :]      # sigma-permuted rows: row p <-> h' SIGMA[p]
    cDi = (-FH.imag)[SIGMA, :]
    consts = {
        "cFh": np.concatenate([F.real[:, :HF], F.imag[:, :HF]], 1).astype(BF16),  # [128, 130]
        "cB1": np.concatenate([RrT, RiT], 1).astype(BF16),                 # [128, 130]
        "cB2": cB2.astype(BF16),                                           # [128, 130]
        "cE1": np.concatenate([S.real, S.imag], 1).astype(BF16),           # [65, 256]
        "cE2": np.concatenate([-S.imag, S.real], 1).astype(BF16),          # [65, 256]
        "cDr": cDr.astype(BF16),                                           # [128, 128]
        "cDi": cDi.astype(BF16),                                           # [128, 128]
    }
    return consts


def _build_program():
    from contextlib import ExitStack

    import concourse.bass as bass  # noqa: F401
    import concourse.mybir as mybir
    import concourse.tile as tile
    from concourse import bacc

    f32 = mybir.dt.float32
    bf = mybir.dt.bfloat16
    AF = mybir.ActivationFunctionType
    ALU = mybir.AluOpType

    nc = bacc.Bacc("TRN2", target_bir_lowering=False, debug=False)

    xh = nc.dram_tensor("xh", [H, C, W], bf, kind="ExternalInput")
    w1k = nc.dram_tensor("w1k", [3, NBL, BS, BS], bf, kind="ExternalInput")
    badd = nc.dram_tensor("badd", [2, NBL, BS], f32, kind="ExternalInput")
    w2a = nc.dram_tensor("w2a", [NBL, BS + 1, 2 * BS], bf, kind="ExternalInput")
    w2b = nc.dram_tensor("w2b", [NBL, BS, 2 * BS], bf, kind="ExternalInput")
    cFh = nc.dram_tensor("cFh", [H, 2 * HF], bf, kind="ExternalInput")
    cB1 = nc.dram_tensor("cB1", [W, 2 * WF], bf, kind="ExternalInput")
    cB2 = nc.dram_tensor("cB2", [W, 2 * WF], bf, kind="ExternalInput")
    cE1 = nc.dram_tensor("cE1", [WF, 2 * W], bf, kind="ExternalInput")
    cE2 = nc.dram_tensor("cE2", [WF, 2 * W], bf, kind="ExternalInput")
    cDr = nc.dram_tensor("cDr", [H, H], bf, kind="ExternalInput")
    cDi = nc.dram_tensor("cDi", [H, H], bf, kind="ExternalInput")
    outs = nc.dram_tensor("outs", [H, C, W], f32, kind="ExternalOutput")

    with ExitStack() as ctx:
        tc = ctx.enter_context(tile.TileContext(nc))
        consts = ctx.enter_context(tc.tile_pool(name="consts", bufs=1))
        xpool = ctx.enter_context(tc.tile_pool(name="xpool", bufs=2))
        zpool = ctx.enter_context(tc.tile_pool(name="zpool", bufs=1))
        wpool = ctx.enter_context(tc.tile_pool(name="wpool", bufs=1))
        ppool = ctx.enter_context(tc.tile_pool(name="ppool", bufs=1))
        mixp = ctx.enter_context(tc.tile_pool(name="mixp", bufs=3))
        outp = ctx.enter_context(tc.tile_pool(name="outp", bufs=3))
        psum = ctx.enter_context(tc.tile_pool(name="psum", bufs=2, space="PSUM"))

        # ---- constants into SBUF ----
        cFh_sb = consts.tile([H, 2 * HF], bf)
        nc.sync.dma_start(cFh_sb, cFh[:])
        cB1_sb = consts.tile([W, 2 * WF], bf)
        nc.sync.dma_start(cB1_sb, cB1[:])
        cB2_sb = consts.tile([W, 2 * WF], bf)
        nc.sync.dma_start(cB2_sb, cB2[:])
        cE1_sb = consts.tile([WF, 2 * W], bf)
        nc.sync.dma_start(cE1_sb, cE1[:])
        cE2_sb = consts.tile([WF, 2 * W], bf)
        nc.sync.dma_start(cE2_sb, cE2[:])
        cDr_sb = consts.tile([H, H], bf)
        nc.sync.dma_start(cDr_sb, cDr[:])
        cDi_sb = consts.tile([H, H], bf)
        nc.sync.dma_start(cDi_sb, cDi[:])

        # layer-1 weights (modulation folded): kinds 0=w1r*sh, 1=-w1i*sh, 2=w1i*sh
        w1_sb = consts.tile([BS, 3, NBL, BS], bf)
        nc.sync.dma_start(w1_sb, w1k[:].rearrange("k n d c -> d k n c"))
        badd_sb = consts.tile([BS, 2, NBL], f32)
        nc.sync.dma_start(badd_sb, badd[:].rearrange("k n d -> d k n"))
        # layer-2 rhs: A=[w2r|W2ri] + bias row, B=[-w2i|W2c]
        w2a_sb = consts.tile([BS + 1, NBL, 2 * BS], bf)
        nc.sync.dma_start(w2a_sb, w2a[:].rearrange("n d c -> d n c"))
        w2b_sb = consts.tile([BS, NBL, 2 * BS], bf)
        nc.sync.dma_start(w2b_sb, w2b[:].rearrange("n d c -> d n c"))
        lamn = consts.tile([128, 1], f32)
        nc.vector.memset(lamn, -LAM)
        zerot = consts.tile([BS, HC, WF], f32)
        nc.vector.memset(zerot, 0.0)

        # ---- main per-block pipeline ----
        for n in range(NBL):
            c0 = n * BS

            X_blk = xpool.tile([H, BS, W], bf, tag="xblk")
            nc.sync.dma_start(X_blk, xh[:, c0 : c0 + BS, :])

            # ---- stage A: Z^T = X_c^T @ [Fr|Fi] (h' 0..64) ----
            Zbuf = zpool.tile([W, BS, 2 * HF], bf, tag="zbuf")
            for cp in range(BS // 2):
                c = 2 * cp
                pA = psum.tile([128, 2, 2 * HF], f32, tag="ps_x")
                nc.tensor.matmul(pA[:, 0, :], lhsT=X_blk[:, c, :], rhs=cFh_sb, start=True, stop=True)
                nc.tensor.matmul(pA[:, 1, :], lhsT=X_blk[:, c + 1, :], rhs=cFh_sb, start=True, stop=True)
                if cp % 2 == 0:
                    nc.vector.tensor_copy(Zbuf[:, c : c + 2, :], pA)
                else:
                    nc.scalar.copy(Zbuf[:, c : c + 2, :], pA)

            # ---- fused B -> l1 -> l2(+shrink) per mirror chunk-pair ----
            # pair k: storage rows lo=[4k,4k+4) hold h'=m, hi=[64+4k,64+4k+4)
            # hold h'=128-m (m=4k+j); spec_lo = T1+T2, spec_hi = T1-T2.
            Wr_pl = wpool.tile([WF, H, BS], bf, tag="wrpl")
            Wi_pl = wpool.tile([WF, H, BS], bf, tag="wipl")
            for k in range(16):
                lo0 = 4 * k
                hi0 = 64 + 4 * k
                arch_lo = mixp.tile([BS, HC, 2 * WF], bf, tag="archl")
                arch_hi = mixp.tile([BS, HC, 2 * WF], bf, tag="archh")
                for j in range(HC):
                    m = 4 * k + j
                    pT = psum.tile([BS, 2, 2 * WF], f32, tag="ps_2")
                    t12 = mixp.tile([BS, 2, 2 * WF], bf, tag="t12")
                    if m == 0:
                        # specials: h'=0 at row 0 (T slot 0), h'=64 at row 64
                        # (T slot 1); both have Zi==0 -> single matmul each.
                        nc.tensor.matmul(pT[:, 0, :], lhsT=Zbuf[:, :, 0], rhs=cB1_sb, start=True, stop=True)
                        nc.tensor.matmul(pT[:, 1, :], lhsT=Zbuf[:, :, 64], rhs=cB1_sb, start=True, stop=True)
                        nc.vector.tensor_copy(t12, pT)
                        nc.gpsimd.tensor_copy(arch_lo[:, 0, :], t12[:, 0, :])
                        nc.gpsimd.tensor_copy(arch_hi[:, 0, :], t12[:, 1, :])
                        continue
                    nc.tensor.matmul(pT[:, 0, :], lhsT=Zbuf[:, :, m], rhs=cB1_sb, start=True, stop=True)
                    nc.tensor.matmul(pT[:, 1, :], lhsT=Zbuf[:, :, HF + m], rhs=cB2_sb, start=True, stop=True)
                    if j % 2 == 0:
                        nc.vector.tensor_copy(t12, pT)
                    else:
                        nc.scalar.copy(t12, pT)
                    nc.gpsimd.tensor_add(arch_lo[:, j, :], t12[:, 0, :], t12[:, 1, :])
                    nc.gpsimd.tensor_sub(arch_hi[:, j, :], t12[:, 0, :], t12[:, 1, :])

                for half, arch, h0 in ((0, arch_lo, lo0), (1, arch_hi, hi0)):
                    Ar = arch[:, :, 0:WF]
                    Ai = arch[:, :, WF : 2 * WF]
                    # layer 1: shared w1r stationary for p1r/p1i first terms
                    p1r = psum.tile([BS, HC, WF], f32, tag="ps_1")
                    nc.tensor.matmul(p1r, lhsT=w1_sb[:, 0, n, :], rhs=Ar, start=True, stop=False)
                    nc.tensor.matmul(p1r, lhsT=w1_sb[:, 1, n, :], rhs=Ai, start=False, stop=True)
                    p1i = psum.tile([BS, HC, WF], f32, tag="ps_1")
                    nc.tensor.matmul(p1i, lhsT=w1_sb[:, 2, n, :], rhs=Ar, start=True, stop=False)
                    nc.tensor.matmul(p1i, lhsT=w1_sb[:, 0, n, :], rhs=Ai, start=False, stop=True)
                    r1a = mixp.tile([BS + 1, HC, WF], bf, tag="r1a")
                    i1 = mixp.tile([BS, HC, WF], bf, tag="i1")
                    nc.scalar.activation(r1a[0:BS], p1r, AF.Relu, bias=badd_sb[:, 0, n : n + 1])
                    nc.gpsimd.memset(r1a[BS : BS + 1, :, :], 1.0)
                    nc.vector.scalar_tensor_tensor(
                        i1, p1i, badd_sb[:, 1, n : n + 1], zerot, ALU.add, ALU.max
                    )
                    # layer 2, data-stationary: out[wf, (r2|i2)] per h'
                    p2 = psum.tile([WF, HC, 256], f32, tag="ps_2")
                    for j in range(HC):
                        nc.tensor.matmul(
                            p2[:, j, 0 : 2 * BS], lhsT=r1a[:, j, :], rhs=w2a_sb[:, n, :],
                            start=True, stop=False,
                        )
                        nc.tensor.matmul(
                            p2[:, j, 0 : 2 * BS], lhsT=i1[:, j, :], rhs=w2b_sb[:, n, :],
                            start=False, stop=True,
                        )
                    # softshrink(v) = relu(v - lam) + min(v + lam, 0)
                    tur = mixp.tile([WF, HC, BS], bf, tag="tur")
                    tmr = mixp.tile([WF, HC, BS], bf, tag="tmr")
                    tui = mixp.tile([WF, HC, BS], bf, tag="tui")
                    tmi = mixp.tile([WF, HC, BS], bf, tag="tmi")
                    nc.scalar.activation(tur, p2[:, :, 0:BS], AF.Relu, bias=lamn[0:WF])
                    nc.vector.tensor_scalar(tmr, p2[:, :, 0:BS], LAM, 0.0, ALU.add, ALU.min)
                    nc.scalar.activation(tui, p2[:, :, BS : 2 * BS], AF.Relu, bias=lamn[0:WF])
                    nc.vector.tensor_scalar(tmi, p2[:, :, BS : 2 * BS], LAM, 0.0, ALU.add, ALU.min)
                    nc.gpsimd.tensor_add(Wr_pl[:, h0 : h0 + HC, :], tur, tmr)
                    nc.gpsimd.tensor_add(Wi_pl[:, h0 : h0 + HC, :], tui, tmi)

            # ---- stage E': [Pr|Pi] = W_c @ [Sr|Si] per channel ----
            Pbuf = ppool.tile([H, BS, 2 * H], bf, tag="pbuf")
            for cp in range(BS // 2):
                c = 2 * cp
                pE = psum.tile([128, 2, 2 * H], f32, tag="ps_x")
                for q in range(2):
                    nc.tensor.matmul(
                        pE[:, q, :], lhsT=Wr_pl[:, :, c + q], rhs=cE1_sb, start=True, stop=False
                    )
                    nc.tensor.matmul(
                        pE[:, q, :], lhsT=Wi_pl[:, :, c + q], rhs=cE2_sb, start=False, stop=True
                    )
                if cp % 2 == 0:
                    nc.vector.tensor_copy(Pbuf[:, c : c + 2, :], pE)
                else:
                    nc.scalar.copy(Pbuf[:, c : c + 2, :], pE)

            # ---- stage D': out = FHrP@Pr + FHiP@Pi + x ----
            for g in range(BS // 4):
                cg0 = g * 4
                pD = psum.tile([H, 4, W], f32, tag="ps_x")
                nc.tensor.matmul(
                    pD, lhsT=cDr_sb, rhs=Pbuf[:, cg0 : cg0 + 4, 0:H], start=True, stop=False
                )
                nc.tensor.matmul(
                    pD, lhsT=cDi_sb, rhs=Pbuf[:, cg0 : cg0 + 4, H : 2 * H], start=False, stop=True
                )
                ot = outp.tile([H, 4, W], f32, tag="ot")
                nc.vector.tensor_add(ot, pD, X_blk[:, cg0 : cg0 + 4, :])
                nc.sync.dma_start(outs[:, c0 + cg0 : c0 + cg0 + 4, :], ot)

    nc.compile()
    return nc


_CACHE = {}


def _get_program():
    if "nc" not in _CACHE:
        _CACHE["nc"] = _build_program()
    return _CACHE["nc"]


def kernel(**inputs):
    x = np.asarray(inputs["x"], dtype=np.float32)
    t = np.asarray(inputs["t"], dtype=np.float32)
    w1 = np.asarray(inputs["w1"], dtype=np.float32)
    b1 = np.asarray(inputs["b1"], dtype=np.float32)
    w2 = np.asarray(inputs["w2"], dtype=np.float32)
    b2 = np.asarray(inputs["b2"], dtype=np.float32)
    mod_w = np.asarray(inputs["mod_w"], dtype=np.float32)
    mod_b = np.asarray(inputs["mod_b"], dtype=np.float32)

    from concourse.bass_utils import run_bass_kernel_spmd

    nc = _get_program()
    consts = _host_consts()

    # adaLN modulation on host: mod = silu(t) @ mod_w.T + mod_b
    st = t / (1.0 + np.exp(-t))
    mod = st @ mod_w.T + mod_b                      # (B, 2*DIM)
    mod = mod.reshape(B_FULL, NB, 2 * BS)
    shift, scale = mod[..., :BS], mod[..., BS:]     # (B, NB, BS)

    in_maps = []
    for core in range(N_CORES):
        b = core // 2
        n0 = (core % 2) * NBL
        cs = slice(n0 * BS, n0 * BS + C)

        # layer-1 weights with modulation folded (column scale); bias vectors
        sh = shift[b, n0 : n0 + NBL] + 1.0          # (NBL, BS) per c_out
        sc = scale[b, n0 : n0 + NBL]
        w1r = w1[0, n0 : n0 + NBL] * sh[:, None, :]  # (NBL, BS, BS)
        w1i = w1[1, n0 : n0 + NBL] * sh[:, None, :]
        addr = b1[0, n0 : n0 + NBL] * sh + sc        # (NBL, BS)
        addi = b1[1, n0 : n0 + NBL] * sh + sc
        w1kk = np.stack([w1r, -w1i, w1i])            # (3, NBL, BS, BS)
        baddk = np.stack([addr, addi])               # (2, NBL, BS)

        # layer-2 composites: r2 = r1@w2r - i1@w2i + b2r
        # i2 = r1@(w2r@w2i) + i1@(w2r - w2i@w2i) + (b2r@w2i + b2i)
        w2r = w2[0, n0 : n0 + NBL]
        w2i = w2[1, n0 : n0 + NBL]
        b2r = b2[0, n0 : n0 + NBL]
        b2i = b2[1, n0 : n0 + NBL]
        W2ri = np.einsum("ndk,nkm->ndm", w2r, w2i)
        W2c = w2r - np.einsum("ndk,nkm->ndm", w2i, w2i)
        b2ip = np.einsum("nk,nkm->nm", b2r, w2i) + b2i
        w2ak = np.concatenate([
            np.concatenate([w2r, W2ri], 2),
            np.concatenate([b2r[:, None, :], b2ip[:, None, :]], 2),
        ], 1)                                        # (NBL, 97, 192)
        w2bk = np.concatenate([-w2i, W2c], 2)        # (NBL, 96, 192)

        im = {
            "xh": np.ascontiguousarray(
                x[b, cs].transpose(1, 0, 2)).astype(BF16),   # [H, C, W]
            "w1k": w1kk.astype(BF16),
            "badd": baddk.astype(np.float32),
            "w2a": w2ak.astype(BF16),
            "w2b": w2bk.astype(BF16),
        }
        im.update(consts)
        in_maps.append(im)

    import os as _os
    trace = bool(int(_os.environ.get("AFNO_TRACE", "0")))
    res = run_bass_kernel_spmd(
        nc, in_maps, core_ids=list(range(N_CORES)), trace=trace
    )
    global LAST_RESULTS
    LAST_RESULTS = res

    out = np.empty((B_FULL, DIM, H, W), dtype=np.float32)
    for core in range(N_CORES):
        b = core // 2
        n0 = (core % 2) * NBL
        cs = slice(n0 * BS, n0 * BS + C)
        out[b, cs] = res.results[core]["outs"].transpose(1, 0, 2)
    return out


# revision 13
# speedup vs baseline: 1.5568x; 1.5568x over previous
"""ModAFNO2D layer as a Bass/Tile kernel for 8 Trainium2 NeuronCores.

Sharding: 8 cores = (batch b in 0..3) x (block-half in 0..1). Each core owns one
batch sample and 4 of the 8 FNO blocks (= 384 of 768 channels). Cores are fully
independent - no collectives; host slices inputs and concatenates outputs.

All matmuls bf16. Host folds the adaLN modulation into the layer-1 weights
(column scaling + bias vectors) and rewrites layer 2 with composite weights so
both its outputs are direct functions of (r1, i1); layer 2 runs data-stationary
(spectrum slice as the PE stationary), landing its output already transposed to
[wf, c] - no explicit PE transposes anywhere.

Stage A exploits Hermitian symmetry of the H-FFT of the real input (h' 0..64
only; stage B mirrors with conjugate weights). Layer 2 packs TWO h' rows per
matmul slot via PE column-groups (wf 0..63 at psum rows 0:64 / 64:128), so the
two N=192 streams run concurrently; the wf=64 (W-Nyquist) column runs in a tiny
side lane whose contribution is a rank-1 (-1)^w correction GA[h,c] folded into
the output evict (the inverse-W DFT row 64 is real: S[64,w] = (-1)^w/128).
Stage E' consumes the parity-packed planes with row/col-group-split matmuls;
the inverse-H DFT matrix is row-permuted on the host to compensate.

Per-core pipeline per block (96 channels):
  A : Zbuf[w, c, (h'r|h'i)] = X_c^T @ [Fr|Fi][:, 0:65]     (FFT along H)
  B : arch[c, hc, (wfr|wfi)] per h'                         (rFFT along W)
  l1: weight-stationary block matmul, modulation pre-folded, relu
  l2: data-stationary, h'-pair col-packed -> psum [wf-pair, (r|i)]; shrink
  E': Pbuf[h'-parity, c, (Pr|Pi)] = W_c @ [Sr|Si]           (inverse rFFT W)
  D': out[h, c, w] = FHrP@Pr + FHiP@Pi + GA*(-1)^w + x      (inverse FFT H)
"""

import numpy as np
import ml_dtypes

BF16 = ml_dtypes.bfloat16

DIM = 768
NB = 8
BS = 96
LAM = 0.01
B_FULL = 4
H = 128
W = 128
WF = W // 2 + 1  # 65
HF = H // 2 + 1  # 65 (Hermitian-reduced H freqs)
WP = 64          # packed wf per column-group half
NBL = 4          # blocks per core
C = NBL * BS     # 384 channels per core
N_CORES = 8
HC = 4           # h' rows per fused B/mix chunk

# E' output row p holds h' PERM2[p]: parity-compact pairs
PERM2 = np.array([2 * p if p < 64 else 2 * (p - 64) + 1 for p in range(H)])


def _host_consts():
    jh = np.arange(H)
    F = np.exp(-2j * np.pi * np.outer(jh, jh) / H)
    R = np.exp(-2j * np.pi * np.outer(np.arange(WF), np.arange(W)) / W) / 128.0
    RrT, RiT = np.ascontiguousarray(R.real.T), np.ascontiguousarray(R.imag.T)
    FH = np.conj(F)
    cw = np.ones(WF)
    cw[1:-1] = 2.0
    S = (cw[:, None] * np.exp(2j * np.pi * np.outer(np.arange(WF), np.arange(W)) / W)) / 128.0
    cB2 = np.concatenate([-RiT, RrT], 1)
    cE1 = np.concatenate([S.real, S.imag], 1)   # [65, 256]
    cE2 = np.concatenate([-S.imag, S.real], 1)
    cE1d = np.concatenate([cE1[0:WP], cE1[0:WP]], 0)   # [128, 256] duplicated
    cE2d = np.concatenate([cE2[0:WP], cE2[0:WP]], 0)
    consts = {
        "cFh": np.concatenate([F.real[:, :HF], F.imag[:, :HF]], 1).astype(BF16),  # [128, 130]
        "cB1": np.concatenate([RrT, RiT], 1).astype(BF16),                 # [128, 130]
        "cB2": cB2.astype(BF16),                                           # [128, 130]
        "cB2n": (-cB2).astype(BF16),                                       # [128, 130]
        "cE1d": cE1d.astype(BF16),                                         # [128, 256]
        "cE2d": cE2d.astype(BF16),                                         # [128, 256]
        "cDr": FH.real[PERM2, :].astype(BF16),                             # [128, 128]
        "cDi": (-FH.imag)[PERM2, :].astype(BF16),                          # [128, 128]
        "cDgr": FH.real.astype(BF16),                                      # [128, 128]
        "cDgi": (-FH.imag).astype(BF16),                                   # [128, 128]
    }
    return consts


def _build_program():
    from contextlib import ExitStack

    import concourse.bass as bass  # noqa: F401
    import concourse.mybir as mybir
    import concourse.tile as tile
    from concourse import bacc

    f32 = mybir.dt.float32
    bf = mybir.dt.bfloat16
    AF = mybir.ActivationFunctionType
    ALU = mybir.AluOpType

    nc = bacc.Bacc("TRN2", target_bir_lowering=False, debug=False)

    xh = nc.dram_tensor("xh", [H, C, W], bf, kind="ExternalInput")
    w1k = nc.dram_tensor("w1k", [3, NBL, BS, BS], bf, kind="ExternalInput")
    badd = nc.dram_tensor("badd", [2, NBL, BS], f32, kind="ExternalInput")
    w2a = nc.dram_tensor("w2a", [NBL, BS + 1, 2 * BS], bf, kind="ExternalInput")
    w2b = nc.dram_tensor("w2b", [NBL, BS, 2 * BS], bf, kind="ExternalInput")
    cFh = nc.dram_tensor("cFh", [H, 2 * HF], bf, kind="ExternalInput")
    cB1 = nc.dram_tensor("cB1", [W, 2 * WF], bf, kind="ExternalInput")
    cB2 = nc.dram_tensor("cB2", [W, 2 * WF], bf, kind="ExternalInput")
    cB2n = nc.dram_tensor("cB2n", [W, 2 * WF], bf, kind="ExternalInput")
    cE1d = nc.dram_tensor("cE1d", [H, 2 * W], bf, kind="ExternalInput")
    cE2d = nc.dram_tensor("cE2d", [H, 2 * W], bf, kind="ExternalInput")
    cDr = nc.dram_tensor("cDr", [H, H], bf, kind="ExternalInput")
    cDi = nc.dram_tensor("cDi", [H, H], bf, kind="ExternalInput")
    cDgr = nc.dram_tensor("cDgr", [H, H], bf, kind="ExternalInput")
    cDgi = nc.dram_tensor("cDgi", [H, H], bf, kind="ExternalInput")
    outs = nc.dram_tensor("outs", [H, C, W], f32, kind="ExternalOutput")

    with ExitStack() as ctx:
        tc = ctx.enter_context(tile.TileContext(nc))
        consts = ctx.enter_context(tc.tile_pool(name="consts", bufs=1))
        xpool = ctx.enter_context(tc.tile_pool(name="xpool", bufs=2))
        zpool = ctx.enter_context(tc.tile_pool(name="zpool", bufs=1))
        wpool = ctx.enter_context(tc.tile_pool(name="wpool", bufs=1))
        ppool = ctx.enter_context(tc.tile_pool(name="ppool", bufs=1))
        mixp = ctx.enter_context(tc.tile_pool(name="mixp", bufs=3))
        outp = ctx.enter_context(tc.tile_pool(name="outp", bufs=3))
        psx = ctx.enter_context(tc.tile_pool(name="psx", bufs=4, space="PSUM"))
        pss = ctx.enter_context(tc.tile_pool(name="pss", bufs=2, space="PSUM"))

        # ---- constants into SBUF ----
        cFh_sb = consts.tile([H, 2 * HF], bf)
        nc.sync.dma_start(cFh_sb, cFh[:])
        cB1_sb = consts.tile([W, 2 * WF], bf)
        nc.sync.dma_start(cB1_sb, cB1[:])
        cB2_sb = consts.tile([W, 2 * WF], bf)
        nc.sync.dma_start(cB2_sb, cB2[:])
        cB2n_sb = consts.tile([W, 2 * WF], bf)
        nc.sync.dma_start(cB2n_sb, cB2n[:])
        cE1d_sb = consts.tile([H, 2 * W], bf)
        nc.sync.dma_start(cE1d_sb, cE1d[:])
        cE2d_sb = consts.tile([H, 2 * W], bf)
        nc.sync.dma_start(cE2d_sb, cE2d[:])
        cDr_sb = consts.tile([H, H], bf)
        nc.sync.dma_start(cDr_sb, cDr[:])
        cDi_sb = consts.tile([H, H], bf)
        nc.sync.dma_start(cDi_sb, cDi[:])
        cDgr_sb = consts.tile([H, H], bf)
        nc.sync.dma_start(cDgr_sb, cDgr[:])
        cDgi_sb = consts.tile([H, H], bf)
        nc.sync.dma_start(cDgi_sb, cDgi[:])

        # layer-1 weights (modulation folded): kinds 0=w1r*sh, 1=-w1i*sh, 2=w1i*sh
        w1_sb = consts.tile([BS, 3, NBL, BS], bf)
        nc.sync.dma_start(w1_sb, w1k[:].rearrange("k n d c -> d k n c"))
        badd_sb = consts.tile([BS, 2, NBL], f32)
        nc.sync.dma_start(badd_sb, badd[:].rearrange("k n d -> d k n"))
        w2a_sb = consts.tile([BS + 1, NBL, 2 * BS], bf)
        nc.sync.dma_start(w2a_sb, w2a[:].rearrange("n d c -> d n c"))
        w2b_sb = consts.tile([BS, NBL, 2 * BS], bf)
        nc.sync.dma_start(w2b_sb, w2b[:].rearrange("n d c -> d n c"))
        lamn = consts.tile([128, 1], f32)
        nc.vector.memset(lamn, -LAM)
        zerot = consts.tile([BS, HC, WF], f32)
        nc.vector.memset(zerot, 0.0)

        # ---- main per-block pipeline ----
        for n in range(NBL):
            c0 = n * BS

            X_blk = xpool.tile([H, BS, W], bf, tag="xblk")
            nc.sync.dma_start(X_blk, xh[:, c0 : c0 + BS, :])

            # ---- stage A: Z^T = X_c^T @ [Fr|Fi] (h' 0..64) ----
            Zbuf = zpool.tile([W, BS, 2 * HF], bf, tag="zbuf")
            for cp in range(BS // 2):
                c = 2 * cp
                pA = psx.tile([128, 2, 2 * HF], f32, tag="ps_x")
                nc.tensor.matmul(pA[:, 0, :], lhsT=X_blk[:, c, :], rhs=cFh_sb, start=True, stop=True)
                nc.tensor.matmul(pA[:, 1, :], lhsT=X_blk[:, c + 1, :], rhs=cFh_sb, start=True, stop=True)
                if cp % 2 == 0:
                    nc.vector.tensor_copy(Zbuf[:, c : c + 2, :], pA)
                else:
                    nc.scalar.copy(Zbuf[:, c : c + 2, :], pA)

            # ---- fused B -> l1 -> l2(+shrink) per chunk of HC h' rows ----
            # packed planes: partition p<64: (wf=p, even h' of pair); p>=64:
            # (wf=p-64, odd h' of pair); pair axis t covers h' (2t, 2t+1)
            Wr_pl = wpool.tile([H, H // 2, BS], bf, tag="wrpl")
            Wi_pl = wpool.tile([H, H // 2, BS], bf, tag="wipl")
            r1ny = wpool.tile([BS + 1, H], bf, tag="r1ny")
            i1ny = wpool.tile([BS, H], bf, tag="i1ny")
            for ch_i in range(H // HC):
                h0 = ch_i * HC
                arch = mixp.tile([BS, HC, 2 * WF], bf, tag="arch")
                for j2 in range(HC // 2):
                    pB = psx.tile([BS, 2, 2 * WF], f32, tag="ps_x")
                    for j in range(2):
                        hj = h0 + j2 * 2 + j
                        m = hj if hj <= 64 else 128 - hj
                        rhs2 = cB2_sb if hj <= 64 else cB2n_sb
                        nc.tensor.matmul(
                            pB[:, j, :], lhsT=Zbuf[:, :, m], rhs=cB1_sb,
                            start=True, stop=False,
                        )
                        nc.tensor.matmul(
                            pB[:, j, :], lhsT=Zbuf[:, :, HF + m], rhs=rhs2,
                            start=False, stop=True,
                        )
                    if j2 % 2 == 0:
                        nc.vector.tensor_copy(arch[:, j2 * 2 : j2 * 2 + 2, :], pB)
                    else:
                        nc.scalar.copy(arch[:, j2 * 2 : j2 * 2 + 2, :], pB)
                Ar = arch[:, :, 0:WF]
                Ai = arch[:, :, WF : 2 * WF]
                # layer 1
                p1r = pss.tile([BS, HC, WF], f32, tag="ps_1")
                nc.tensor.matmul(p1r, lhsT=w1_sb[:, 0, n, :], rhs=Ar, start=True, stop=False)
                nc.tensor.matmul(p1r, lhsT=w1_sb[:, 1, n, :], rhs=Ai, start=False, stop=True)
                p1i = pss.tile([BS, HC, WF], f32, tag="ps_1")
                nc.tensor.matmul(p1i, lhsT=w1_sb[:, 2, n, :], rhs=Ar, start=True, stop=False)
                nc.tensor.matmul(p1i, lhsT=w1_sb[:, 0, n, :], rhs=Ai, start=False, stop=True)
                r1a = mixp.tile([BS + 1, HC, WF], bf, tag="r1a")
                i1 = mixp.tile([BS, HC, WF], bf, tag="i1")
                nc.scalar.activation(r1a[0:BS], p1r, AF.Relu, bias=badd_sb[:, 0, n : n + 1])
                nc.gpsimd.memset(r1a[BS : BS + 1, :, :], 1.0)
                nc.vector.scalar_tensor_tensor(
                    i1, p1i, badd_sb[:, 1, n : n + 1], zerot, ALU.add, ALU.max
                )
                # stash the wf=64 (W-Nyquist) columns for the side lane
                nc.gpsimd.tensor_copy(r1ny[:, h0 : h0 + HC], r1a[:, :, WP])
                nc.gpsimd.tensor_copy(i1ny[:, h0 : h0 + HC], i1[:, :, WP])
                # layer 2, data-stationary, h'-pair col-packed (wf 0..63)
                p2 = pss.tile([128, 2, 256], f32, tag="ps_2")
                for q in range(2):
                    ja, jb = 2 * q, 2 * q + 1
                    nc.tensor.matmul(
                        p2[0:WP, q, 0 : 2 * BS], lhsT=r1a[:, ja, 0:WP], rhs=w2a_sb[:, n, :],
                        start=True, stop=False,
                    )
                    nc.tensor.matmul(
                        p2[WP:128, q, 0 : 2 * BS], lhsT=r1a[:, jb, 0:WP], rhs=w2a_sb[:, n, :],
                        start=True, stop=False,
                    )
                    nc.tensor.matmul(
                        p2[0:WP, q, 0 : 2 * BS], lhsT=i1[:, ja, 0:WP], rhs=w2b_sb[:, n, :],
                        start=False, stop=False,
                    )
                    nc.tensor.matmul(
                        p2[WP:128, q, 0 : 2 * BS], lhsT=i1[:, jb, 0:WP], rhs=w2b_sb[:, n, :],
                        start=False, stop=True,
                    )
                # softshrink(v) = relu(v - lam) + min(v + lam, 0)
                tur = mixp.tile([128, 2, BS], bf, tag="tur")
                tmr = mixp.tile([128, 2, BS], bf, tag="tmr")
                tui = mixp.tile([128, 2, BS], bf, tag="tui")
                tmi = mixp.tile([128, 2, BS], bf, tag="tmi")
                nc.scalar.activation(tur, p2[:, :, 0:BS], AF.Relu, bias=lamn)
                nc.vector.tensor_scalar(tmr, p2[:, :, 0:BS], LAM, 0.0, ALU.add, ALU.min)
                nc.scalar.activation(tui, p2[:, :, BS : 2 * BS], AF.Relu, bias=lamn)
                nc.vector.tensor_scalar(tmi, p2[:, :, BS : 2 * BS], LAM, 0.0, ALU.add, ALU.min)
                nc.vector.tensor_add(Wr_pl[:, 2 * ch_i : 2 * ch_i + 2, :], tur, tmr)
                nc.gpsimd.tensor_add(Wi_pl[:, 2 * ch_i : 2 * ch_i + 2, :], tui, tmi)

            # ---- W-Nyquist side lane: l2 on wf=64 column, all 128 h' ----
            pny = pss.tile([128, 2 * BS], f32, tag="ps_1")
            for g in range(4):
                r0 = 32 * g
                nc.tensor.matmul(
                    pny[r0 : r0 + 32, :], lhsT=r1ny[:, r0 : r0 + 32], rhs=w2a_sb[:, n, :],
                    start=True, stop=False, tile_position=(0, r0),
                )
                nc.tensor.matmul(
                    pny[r0 : r0 + 32, :], lhsT=i1ny[:, r0 : r0 + 32], rhs=w2b_sb[:, n, :],
                    start=False, stop=True, tile_position=(0, r0),
                )
            nur = mixp.tile([128, BS], bf, tag="nur")
            nmr = mixp.tile([128, BS], bf, tag="nmr")
            nui = mixp.tile([128, BS], bf, tag="nui")
            nmi = mixp.tile([128, BS], bf, tag="nmi")
            Wny_r = wpool.tile([128, BS], bf, tag="wnyr")
            Wny_i = wpool.tile([128, BS], bf, tag="wnyi")
            nc.scalar.activation(nur, pny[:, 0:BS], AF.Relu, bias=lamn)
            nc.vector.tensor_scalar(nmr, pny[:, 0:BS], LAM, 0.0, ALU.add, ALU.min)
            nc.scalar.activation(nui, pny[:, BS : 2 * BS], AF.Relu, bias=lamn)
            nc.vector.tensor_scalar(nmi, pny[:, BS : 2 * BS], LAM, 0.0, ALU.add, ALU.min)
            nc.vector.tensor_add(Wny_r, nur, nmr)
            nc.gpsimd.tensor_add(Wny_i, nui, nmi)
            # GA[h, c] = sum_h' FHr[h', h] Wny_r + FHi'[h', h] Wny_i (scaled 1/128)
            pGA = pss.tile([128, BS], f32, tag="ps_1")
            nc.tensor.matmul(pGA, lhsT=cDgr_sb, rhs=Wny_r, start=True, stop=False)
            nc.tensor.matmul(pGA, lhsT=cDgi_sb, rhs=Wny_i, start=False, stop=True)
            GA_sb = wpool.tile([128, BS], bf, tag="ga")
            nc.vector.tensor_scalar_mul(GA_sb, pGA, 1.0 / 128.0)

            # ---- stage E': [Pr|Pi] = W_c @ [Sr|Si], parity-split matmuls ----
            Pbuf = ppool.tile([H, BS, 2 * H], bf, tag="pbuf")
            for cp in range(BS // 2):
                c = 2 * cp
                pE = psx.tile([128, 2, 2 * H], f32, tag="ps_x")
                for q in range(2):
                    nc.tensor.matmul(
                        pE[0:WP, q, :], lhsT=Wr_pl[0:WP, :, c + q], rhs=cE1d_sb[0:WP, :],
                        start=True, stop=False,
                    )
                    nc.tensor.matmul(
                        pE[WP:128, q, :], lhsT=Wr_pl[WP:128, :, c + q], rhs=cE1d_sb[WP:128, :],
                        start=True, stop=False,
                    )
                    nc.tensor.matmul(
                        pE[0:WP, q, :], lhsT=Wi_pl[0:WP, :, c + q], rhs=cE2d_sb[0:WP, :],
                        start=False, stop=False,
                    )
                    nc.tensor.matmul(
                        pE[WP:128, q, :], lhsT=Wi_pl[WP:128, :, c + q], rhs=cE2d_sb[WP:128, :],
                        start=False, stop=True,
                    )
                if cp % 2 == 0:
                    nc.vector.tensor_copy(Pbuf[:, c : c + 2, :], pE)
                else:
                    nc.scalar.copy(Pbuf[:, c : c + 2, :], pE)

            # ---- stage D': out = FHrP@Pr + FHiP@Pi + GA*(-1)^w + x ----
            for g in range(BS // 4):
                cg0 = g * 4
                pD = psx.tile([H, 4, W], f32, tag="ps_x")
                nc.tensor.matmul(
                    pD, lhsT=cDr_sb, rhs=Pbuf[:, cg0 : cg0 + 4, 0:H], start=True, stop=False
                )
                nc.tensor.matmul(
                    pD, lhsT=cDi_sb, rhs=Pbuf[:, cg0 : cg0 + 4, H : 2 * H], start=False, stop=True
                )
                ot = outp.tile([H, 4, W], f32, tag="ot")
                nc.vector.tensor_add(ot, pD, X_blk[:, cg0 : cg0 + 4, :])
                GAb = GA_sb[:, cg0 : cg0 + 4].broadcast_to((H, 4, W // 2))
                nc.gpsimd.tensor_add(ot[:, :, 0:W:2], ot[:, :, 0:W:2], GAb)
                nc.gpsimd.tensor_sub(ot[:, :, 1:W:2], ot[:, :, 1:W:2], GAb)
                nc.sync.dma_start(outs[:, c0 + cg0 : c0 + cg0 + 4, :], ot)

    nc.compile()
    return nc


_CACHE = {}


def _get_program():
    if "nc" not in _CACHE:
        _CACHE["nc"] = _build_program()
    return _CACHE["nc"]


def kernel(**inputs):
    x = np.asarray(inputs["x"], dtype=np.float32)
    t = np.asarray(inputs["t"], dtype=np.float32)
    w1 = np.asarray(inputs["w1"], dtype=np.float32)
    b1 = np.asarray(inputs["b1"], dtype=np.float32)
    w2 = np.asarray(inputs["w2"], dtype=np.float32)
    b2 = np.asarray(inputs["b2"], dtype=np.float32)
    mod_w = np.asarray(inputs["mod_w"], dtype=np.float32)
    mod_b = np.asarray(inputs["mod_b"], dtype=np.float32)

    from concourse.bass_utils import run_bass_kernel_spmd

    nc = _get_program()
    consts = _host_consts()

    # adaLN modulation on host: mod = silu(t) @ mod_w.T + mod_b
    st = t / (1.0 + np.exp(-t))
    mod = st @ mod_w.T + mod_b                      # (B, 2*DIM)
    mod = mod.reshape(B_FULL, NB, 2 * BS)
    shift, scale = mod[..., :BS], mod[..., BS:]     # (B, NB, BS)

    in_maps = []
    for core in range(N_CORES):
        b = core // 2
        n0 = (core % 2) * NBL
        cs = slice(n0 * BS, n0 * BS + C)

        sh = shift[b, n0 : n0 + NBL] + 1.0          # (NBL, BS) per c_out
        sc = scale[b, n0 : n0 + NBL]
        w1r = w1[0, n0 : n0 + NBL] * sh[:, None, :]  # (NBL, BS, BS)
        w1i = w1[1, n0 : n0 + NBL] * sh[:, None, :]
        addr = b1[0, n0 : n0 + NBL] * sh + sc        # (NBL, BS)
        addi = b1[1, n0 : n0 + NBL] * sh + sc
        w1kk = np.stack([w1r, -w1i, w1i])            # (3, NBL, BS, BS)
        baddk = np.stack([addr, addi])               # (2, NBL, BS)

        # layer-2 composites: r2 = r1@w2r - i1@w2i + b2r
        # i2 = r1@(w2r@w2i) + i1@(w2r - w2i@w2i) + (b2r@w2i + b2i)
        w2r = w2[0, n0 : n0 + NBL]
        w2i = w2[1, n0 : n0 + NBL]
        b2r = b2[0, n0 : n0 + NBL]
        b2i = b2[1, n0 : n0 + NBL]
        W2ri = np.einsum("ndk,nkm->ndm", w2r, w2i)
        W2c = w2r - np.einsum("ndk,nkm->ndm", w2i, w2i)
        b2ip = np.einsum("nk,nkm->nm", b2r, w2i) + b2i
        w2ak = np.concatenate([
            np.concatenate([w2r, W2ri], 2),
            np.concatenate([b2r[:, None, :], b2ip[:, None, :]], 2),
        ], 1)                                        # (NBL, 97, 192)
        w2bk = np.concatenate([-w2i, W2c], 2)        # (NBL, 96, 192)

        im = {
            "xh": np.ascontiguousarray(
                x[b, cs].transpose(1, 0, 2)).astype(BF16),   # [H, C, W]
            "w1k": w1kk.astype(BF16),
            "badd": baddk.astype(np.float32),
            "w2a": w2ak.astype(BF16),
            "w2b": w2bk.astype(BF16),
        }
        im.update(consts)
        in_maps.append(im)

    import os as _os
    trace = bool(int(_os.environ.get("AFNO_TRACE", "0")))
    res = run_bass_kernel_spmd(
        nc, in_maps, core_ids=list(range(N_CORES)), trace=trace
    )
    global LAST_RESULTS
    LAST_RESULTS = res

    out = np.empty((B_FULL, DIM, H, W), dtype=np.float32)
    for core in range(N_CORES):
        b = core // 2
        n0 = (core % 2) * NBL
        cs = slice(n0 * BS, n0 * BS + C)
        out[b, cs] = res.results[core]["outs"].transpose(1, 0, 2)
    return out


# revision 14
# speedup vs baseline: 2.0696x; 1.3294x over previous
"""ModAFNO2D layer as a Bass/Tile kernel for 8 Trainium2 NeuronCores.

Sharding: 8 cores = (batch b in 0..3) x (block-half in 0..1). Each core owns one
batch sample and 4 of the 8 FNO blocks (= 384 of 768 channels). Cores are fully
independent - no collectives; host slices inputs and concatenates outputs.

All matmuls bf16. Host folds the adaLN modulation into the layer-1 weights
(column scaling + bias vectors) and rewrites layer 2 with composite weights so
both its outputs are direct functions of (r1, i1); layer 2 runs data-stationary
(spectrum slice as the PE stationary), landing its output already transposed to
[wf, c] - no explicit PE transposes anywhere.

Stage A exploits Hermitian symmetry of the H-FFT of the real input (h' 0..64
only; stage B mirrors with conjugate weights). Layer 2 packs TWO h' rows per
matmul slot via PE column-groups (wf 0..63 at psum rows 0:64 / 64:128), so the
two N=192 streams run concurrently; the wf=64 (W-Nyquist) column runs in a tiny
side lane whose contribution is a rank-1 (-1)^w correction GA[h,c] folded into
the output evict (the inverse-W DFT row 64 is real: S[64,w] = (-1)^w/128).
Stage E' consumes the parity-packed planes with row/col-group-split matmuls;
the inverse-H DFT matrix is row-permuted on the host to compensate.

Per-core pipeline per block (96 channels):
  A : Zbuf[w, c, (h'r|h'i)] = X_c^T @ [Fr|Fi][:, 0:65]     (FFT along H)
  B : arch[c, hc, (wfr|wfi)] per h'                         (rFFT along W)
  l1: weight-stationary block matmul, modulation pre-folded, relu
  l2: data-stationary, h'-pair col-packed -> psum [wf-pair, (r|i)]; shrink
  E': Pbuf[h'-parity, c, (Pr|Pi)] = W_c @ [Sr|Si]           (inverse rFFT W)
  D': out[h, c, w] = FHrP@Pr + FHiP@Pi + GA*(-1)^w + x      (inverse FFT H)
"""

import numpy as np
import ml_dtypes

BF16 = ml_dtypes.bfloat16

DIM = 768
NB = 8
BS = 96
LAM = 0.01
B_FULL = 4
H = 128
W = 128
WF = W // 2 + 1  # 65
HF = H // 2 + 1  # 65 (Hermitian-reduced H freqs)
WP = 64          # packed wf per column-group half
NBL = 4          # blocks per core
C = NBL * BS     # 384 channels per core
N_CORES = 8
HC = 4           # h' rows per fused B/mix chunk

def _host_consts():
    jh = np.arange(H)
    F = np.exp(-2j * np.pi * np.outer(jh, jh) / H)
    R = np.exp(-2j * np.pi * np.outer(np.arange(WF), np.arange(W)) / W) / 128.0
    RrT, RiT = np.ascontiguousarray(R.real.T), np.ascontiguousarray(R.imag.T)
    FH = np.conj(F)
    cw = np.ones(WF)
    cw[1:-1] = 2.0
    S = (cw[:, None] * np.exp(2j * np.pi * np.outer(np.arange(WF), np.arange(W)) / W)) / 128.0
    cB2 = np.concatenate([-RiT, RrT], 1)
    cE1 = np.concatenate([S.real, S.imag], 1)   # [65, 256]
    cE2 = np.concatenate([-S.imag, S.real], 1)
    cEs = np.concatenate([cE1[0:WP], cE2[0:WP]], 0)    # [128, 256] R-top I-bottom
    consts = {
        "cFh": np.concatenate([F.real[:, :HF], F.imag[:, :HF]], 1).astype(BF16),  # [128, 130]
        "cB1": np.concatenate([RrT, RiT], 1).astype(BF16),                 # [128, 130]
        "cB2": cB2.astype(BF16),                                           # [128, 130]
        "cB2n": (-cB2).astype(BF16),                                       # [128, 130]
        "cEs": cEs.astype(BF16),                                           # [128, 256]
        "cDr": FH.real.astype(BF16),                                       # [128, 128]
        "cDi": (-FH.imag).astype(BF16),                                    # [128, 128]
    }
    return consts


def _build_program():
    from contextlib import ExitStack

    import concourse.bass as bass  # noqa: F401
    import concourse.mybir as mybir
    import concourse.tile as tile
    from concourse import bacc

    f32 = mybir.dt.float32
    bf = mybir.dt.bfloat16
    AF = mybir.ActivationFunctionType
    ALU = mybir.AluOpType

    nc = bacc.Bacc("TRN2", target_bir_lowering=False, debug=False)

    xh = nc.dram_tensor("xh", [H, C, W], bf, kind="ExternalInput")
    w1k = nc.dram_tensor("w1k", [3, NBL, BS, BS], bf, kind="ExternalInput")
    badd = nc.dram_tensor("badd", [2, NBL, BS], f32, kind="ExternalInput")
    w2a = nc.dram_tensor("w2a", [NBL, BS + 1, 2 * BS], bf, kind="ExternalInput")
    w2b = nc.dram_tensor("w2b", [NBL, BS, 2 * BS], bf, kind="ExternalInput")
    cFh = nc.dram_tensor("cFh", [H, 2 * HF], bf, kind="ExternalInput")
    cB1 = nc.dram_tensor("cB1", [W, 2 * WF], bf, kind="ExternalInput")
    cB2 = nc.dram_tensor("cB2", [W, 2 * WF], bf, kind="ExternalInput")
    cB2n = nc.dram_tensor("cB2n", [W, 2 * WF], bf, kind="ExternalInput")
    cEs = nc.dram_tensor("cEs", [H, 2 * W], bf, kind="ExternalInput")
    cDr = nc.dram_tensor("cDr", [H, H], bf, kind="ExternalInput")
    cDi = nc.dram_tensor("cDi", [H, H], bf, kind="ExternalInput")
    outs = nc.dram_tensor("outs", [H, C, W], f32, kind="ExternalOutput")

    with ExitStack() as ctx:
        tc = ctx.enter_context(tile.TileContext(nc))
        consts = ctx.enter_context(tc.tile_pool(name="consts", bufs=1))
        xpool = ctx.enter_context(tc.tile_pool(name="xpool", bufs=2))
        zpool = ctx.enter_context(tc.tile_pool(name="zpool", bufs=1))
        wpool = ctx.enter_context(tc.tile_pool(name="wpool", bufs=1))
        ppool = ctx.enter_context(tc.tile_pool(name="ppool", bufs=1))
        mixp = ctx.enter_context(tc.tile_pool(name="mixp", bufs=3))
        outp = ctx.enter_context(tc.tile_pool(name="outp", bufs=3))
        psx = ctx.enter_context(tc.tile_pool(name="psx", bufs=4, space="PSUM"))
        pss = ctx.enter_context(tc.tile_pool(name="pss", bufs=2, space="PSUM"))

        # ---- constants into SBUF ----
        cFh_sb = consts.tile([H, 2 * HF], bf)
        nc.sync.dma_start(cFh_sb, cFh[:])
        cB1_sb = consts.tile([W, 2 * WF], bf)
        nc.sync.dma_start(cB1_sb, cB1[:])
        cB2_sb = consts.tile([W, 2 * WF], bf)
        nc.sync.dma_start(cB2_sb, cB2[:])
        cB2n_sb = consts.tile([W, 2 * WF], bf)
        nc.sync.dma_start(cB2n_sb, cB2n[:])
        cEs_sb = consts.tile([H, 2 * W], bf)
        nc.sync.dma_start(cEs_sb, cEs[:])
        cDr_sb = consts.tile([H, H], bf)
        nc.sync.dma_start(cDr_sb, cDr[:])
        cDi_sb = consts.tile([H, H], bf)
        nc.sync.dma_start(cDi_sb, cDi[:])

        # layer-1 weights (modulation folded): kinds 0=w1r*sh, 1=-w1i*sh, 2=w1i*sh
        w1_sb = consts.tile([BS, 3, NBL, BS], bf)
        nc.sync.dma_start(w1_sb, w1k[:].rearrange("k n d c -> d k n c"))
        badd_sb = consts.tile([BS, 2, NBL], f32)
        nc.sync.dma_start(badd_sb, badd[:].rearrange("k n d -> d k n"))
        w2a_sb = consts.tile([BS + 1, NBL, 2 * BS], bf)
        nc.sync.dma_start(w2a_sb, w2a[:].rearrange("n d c -> d n c"))
        w2b_sb = consts.tile([BS, NBL, 2 * BS], bf)
        nc.sync.dma_start(w2b_sb, w2b[:].rearrange("n d c -> d n c"))
        lamn = consts.tile([128, 1], f32)
        nc.vector.memset(lamn, -LAM)
        zerot = consts.tile([BS, HC, WF], f32)
        nc.vector.memset(zerot, 0.0)

        # ---- main per-block pipeline ----
        for n in range(NBL):
            c0 = n * BS

            X_blk = xpool.tile([H, BS, W], bf, tag="xblk")
            nc.sync.dma_start(X_blk, xh[:, c0 : c0 + BS, :])

            # ---- stage A: Z^T = X_c^T @ [Fr|Fi] (h' 0..64) ----
            Zbuf = zpool.tile([W, 128, 2 * HF], bf, tag="zbuf")
            for cp in range(BS // 2):
                c = 2 * cp
                pA = psx.tile([128, 2, 2 * HF], f32, tag="ps_x")
                nc.tensor.matmul(pA[:, 0, :], lhsT=X_blk[:, c, :], rhs=cFh_sb, start=True, stop=True)
                nc.tensor.matmul(pA[:, 1, :], lhsT=X_blk[:, c + 1, :], rhs=cFh_sb, start=True, stop=True)
                if cp % 2 == 0:
                    nc.vector.tensor_copy(Zbuf[:, c : c + 2, :], pA)
                else:
                    nc.scalar.copy(Zbuf[:, c : c + 2, :], pA)

            # ---- fused B -> l1 -> l2(+shrink) per chunk of HC h' rows ----
            # packed planes: partition p<64: (wf=p, even h' of pair); p>=64:
            # (wf=p-64, odd h' of pair); pair axis t covers h' (2t, 2t+1)
            Wsh = wpool.tile([H, H, BS], bf, tag="wsh")
            r1ny = wpool.tile([BS + 1, H], bf, tag="r1ny")
            i1ny = wpool.tile([BS, H], bf, tag="i1ny")
            for ch_i in range(H // HC):
                h0 = ch_i * HC
                arch = mixp.tile([BS, HC, 2 * WF], bf, tag="arch")
                for j2 in range(HC // 2):
                    pB = psx.tile([128, 2, 2 * WF], f32, tag="ps_x")
                    for j in range(2):
                        hj = h0 + j2 * 2 + j
                        m = hj if hj <= 64 else 128 - hj
                        rhs2 = cB2_sb if hj <= 64 else cB2n_sb
                        nc.tensor.matmul(
                            pB[:, j, :], lhsT=Zbuf[:, :, m], rhs=cB1_sb,
                            start=True, stop=False,
                        )
                        nc.tensor.matmul(
                            pB[:, j, :], lhsT=Zbuf[:, :, HF + m], rhs=rhs2,
                            start=False, stop=True,
                        )
                    if j2 % 2 == 0:
                        nc.vector.tensor_copy(arch[:, j2 * 2 : j2 * 2 + 2, :], pB[0:BS])
                    else:
                        nc.scalar.copy(arch[:, j2 * 2 : j2 * 2 + 2, :], pB[0:BS])
                Ar = arch[:, :, 0:WF]
                Ai = arch[:, :, WF : 2 * WF]
                # layer 1
                p1r = pss.tile([BS, HC, WF], f32, tag="ps_1")
                nc.tensor.matmul(p1r, lhsT=w1_sb[:, 0, n, :], rhs=Ar, start=True, stop=False)
                nc.tensor.matmul(p1r, lhsT=w1_sb[:, 1, n, :], rhs=Ai, start=False, stop=True)
                p1i = pss.tile([BS, HC, WF], f32, tag="ps_1")
                nc.tensor.matmul(p1i, lhsT=w1_sb[:, 2, n, :], rhs=Ar, start=True, stop=False)
                nc.tensor.matmul(p1i, lhsT=w1_sb[:, 0, n, :], rhs=Ai, start=False, stop=True)
                r1a = mixp.tile([BS + 1, HC, WF], bf, tag="r1a")
                i1 = mixp.tile([BS, HC, WF], bf, tag="i1")
                nc.scalar.activation(r1a[0:BS], p1r, AF.Relu, bias=badd_sb[:, 0, n : n + 1])
                nc.gpsimd.memset(r1a[BS : BS + 1, :, :], 1.0)
                nc.vector.scalar_tensor_tensor(
                    i1, p1i, badd_sb[:, 1, n : n + 1], zerot, ALU.add, ALU.max
                )
                # stash the wf=64 (W-Nyquist) columns for the side lane
                nc.gpsimd.tensor_copy(r1ny[:, h0 : h0 + HC], r1a[:, :, WP])
                nc.gpsimd.tensor_copy(i1ny[:, h0 : h0 + HC], i1[:, :, WP])
                # layer 2, data-stationary, vertical R|I split (wf 0..63):
                # psum rows 0:64 = r2(wf), rows 64:128 = i2(wf) per h'
                p2 = pss.tile([128, HC, BS], f32, tag="ps_2")
                for j in range(HC):
                    nc.tensor.matmul(
                        p2[0:WP, j, :], lhsT=r1a[:, j, 0:WP], rhs=w2a_sb[:, n, 0:BS],
                        start=True, stop=False,
                    )
                    nc.tensor.matmul(
                        p2[WP:128, j, :], lhsT=r1a[:, j, 0:WP], rhs=w2a_sb[:, n, BS : 2 * BS],
                        start=True, stop=False,
                    )
                    nc.tensor.matmul(
                        p2[0:WP, j, :], lhsT=i1[:, j, 0:WP], rhs=w2b_sb[:, n, 0:BS],
                        start=False, stop=True,
                    )
                    nc.tensor.matmul(
                        p2[WP:128, j, :], lhsT=i1[:, j, 0:WP], rhs=w2b_sb[:, n, BS : 2 * BS],
                        start=False, stop=True,
                    )
                # softshrink(v) = relu(v - lam) + min(v + lam, 0), both planes at once
                tu = mixp.tile([128, HC, BS], bf, tag="tu")
                tm = mixp.tile([128, HC, BS], bf, tag="tm")
                nc.scalar.activation(tu, p2, AF.Relu, bias=lamn)
                nc.vector.tensor_scalar(tm, p2, LAM, 0.0, ALU.add, ALU.min)
                if ch_i % 2 == 0:
                    nc.vector.tensor_add(Wsh[:, h0 : h0 + HC, :], tu, tm)
                else:
                    nc.gpsimd.tensor_add(Wsh[:, h0 : h0 + HC, :], tu, tm)

            # ---- W-Nyquist side lane: l2 on wf=64 column, all 128 h' ----
            pny = pss.tile([128, 2 * BS], f32, tag="ps_1")
            for g in range(4):
                r0 = 32 * g
                nc.tensor.matmul(
                    pny[r0 : r0 + 32, :], lhsT=r1ny[:, r0 : r0 + 32], rhs=w2a_sb[:, n, :],
                    start=True, stop=False, tile_position=(0, r0),
                )
                nc.tensor.matmul(
                    pny[r0 : r0 + 32, :], lhsT=i1ny[:, r0 : r0 + 32], rhs=w2b_sb[:, n, :],
                    start=False, stop=True, tile_position=(0, r0),
                )
            nur = mixp.tile([128, BS], bf, tag="nur")
            nmr = mixp.tile([128, BS], bf, tag="nmr")
            nui = mixp.tile([128, BS], bf, tag="nui")
            nmi = mixp.tile([128, BS], bf, tag="nmi")
            Wny_r = wpool.tile([128, BS], bf, tag="wnyr")
            Wny_i = wpool.tile([128, BS], bf, tag="wnyi")
            nc.scalar.activation(nur, pny[:, 0:BS], AF.Relu, bias=lamn)
            nc.vector.tensor_scalar(nmr, pny[:, 0:BS], LAM, 0.0, ALU.add, ALU.min)
            nc.scalar.activation(nui, pny[:, BS : 2 * BS], AF.Relu, bias=lamn)
            nc.vector.tensor_scalar(nmi, pny[:, BS : 2 * BS], LAM, 0.0, ALU.add, ALU.min)
            nc.vector.tensor_add(Wny_r, nur, nmr)
            nc.gpsimd.tensor_add(Wny_i, nui, nmi)
            # GA[h, c] = sum_h' FHr[h', h] Wny_r + FHi'[h', h] Wny_i (scaled 1/128)
            pGA = pss.tile([128, BS], f32, tag="ps_1")
            nc.tensor.matmul(pGA, lhsT=cDr_sb, rhs=Wny_r, start=True, stop=False)
            nc.tensor.matmul(pGA, lhsT=cDi_sb, rhs=Wny_i, start=False, stop=True)
            GA_sb = wpool.tile([128, BS], bf, tag="ga")
            nc.vector.tensor_scalar_mul(GA_sb, pGA, 1.0 / 128.0)

            # ---- stage E': one K=128 matmul per channel (R|I stacked) ----
            Pbuf = ppool.tile([H, BS, 2 * H], bf, tag="pbuf")
            for cp in range(BS // 2):
                c = 2 * cp
                pE = psx.tile([128, 2, 2 * H], f32, tag="ps_x")
                for q in range(2):
                    nc.tensor.matmul(
                        pE[:, q, :], lhsT=Wsh[:, :, c + q], rhs=cEs_sb,
                        start=True, stop=True,
                    )
                if cp % 2 == 0:
                    nc.vector.tensor_copy(Pbuf[:, c : c + 2, :], pE)
                else:
                    nc.scalar.copy(Pbuf[:, c : c + 2, :], pE)

            # ---- stage D': out = FHrP@Pr + FHiP@Pi + GA*(-1)^w + x ----
            for g in range(BS // 4):
                cg0 = g * 4
                pD = psx.tile([H, 4, W], f32, tag="ps_x")
                nc.tensor.matmul(
                    pD, lhsT=cDr_sb, rhs=Pbuf[:, cg0 : cg0 + 4, 0:H], start=True, stop=False
                )
                nc.tensor.matmul(
                    pD, lhsT=cDi_sb, rhs=Pbuf[:, cg0 : cg0 + 4, H : 2 * H], start=False, stop=True
                )
                ot = outp.tile([H, 4, W], f32, tag="ot")
                nc.vector.tensor_add(ot, pD, X_blk[:, cg0 : cg0 + 4, :])
                GAb = GA_sb[:, cg0 : cg0 + 4].broadcast_to((H, 4, W // 2))
                nc.gpsimd.tensor_add(ot[:, :, 0:W:2], ot[:, :, 0:W:2], GAb)
                nc.gpsimd.tensor_sub(ot[:, :, 1:W:2], ot[:, :, 1:W:2], GAb)
                nc.sync.dma_start(outs[:, c0 + cg0 : c0 + cg0 + 4, :], ot)

    nc.compile()
    return nc


_CACHE = {}


def _get_program():
    if "nc" not in _CACHE:
        _CACHE["nc"] = _build_program()
    return _CACHE["nc"]


def kernel(**inputs):
    x = np.asarray(inputs["x"], dtype=np.float32)
    t = np.asarray(inputs["t"], dtype=np.float32)
    w1 = np.asarray(inputs["w1"], dtype=np.float32)
    b1 = np.asarray(inputs["b1"], dtype=np.float32)
    w2 = np.asarray(inputs["w2"], dtype=np.float32)
    b2 = np.asarray(inputs["b2"], dtype=np.float32)
    mod_w = np.asarray(inputs["mod_w"], dtype=np.float32)
    mod_b = np.asarray(inputs["mod_b"], dtype=np.float32)

    from concourse.bass_utils import run_bass_kernel_spmd

    nc = _get_program()
    consts = _host_consts()

    # adaLN modulation on host: mod = silu(t) @ mod_w.T + mod_b
    st = t / (1.0 + np.exp(-t))
    mod = st @ mod_w.T + mod_b                      # (B, 2*DIM)
    mod = mod.reshape(B_FULL, NB, 2 * BS)
    shift, scale = mod[..., :BS], mod[..., BS:]     # (B, NB, BS)

    in_maps = []
    for core in range(N_CORES):
        b = core // 2
        n0 = (core % 2) * NBL
        cs = slice(n0 * BS, n0 * BS + C)

        sh = shift[b, n0 : n0 + NBL] + 1.0          # (NBL, BS) per c_out
        sc = scale[b, n0 : n0 + NBL]
        w1r = w1[0, n0 : n0 + NBL] * sh[:, None, :]  # (NBL, BS, BS)
        w1i = w1[1, n0 : n0 + NBL] * sh[:, None, :]
        addr = b1[0, n0 : n0 + NBL] * sh + sc        # (NBL, BS)
        addi = b1[1, n0 : n0 + NBL] * sh + sc
        w1kk = np.stack([w1r, -w1i, w1i])            # (3, NBL, BS, BS)
        baddk = np.stack([addr, addi])               # (2, NBL, BS)

        # layer-2 composites: r2 = r1@w2r - i1@w2i + b2r
        # i2 = r1@(w2r@w2i) + i1@(w2r - w2i@w2i) + (b2r@w2i + b2i)
        w2r = w2[0, n0 : n0 + NBL]
        w2i = w2[1, n0 : n0 + NBL]
        b2r = b2[0, n0 : n0 + NBL]
        b2i = b2[1, n0 : n0 + NBL]
        W2ri = np.einsum("ndk,nkm->ndm", w2r, w2i)
        W2c = w2r - np.einsum("ndk,nkm->ndm", w2i, w2i)
        b2ip = np.einsum("nk,nkm->nm", b2r, w2i) + b2i
        w2ak = np.concatenate([
            np.concatenate([w2r, W2ri], 2),
            np.concatenate([b2r[:, None, :], b2ip[:, None, :]], 2),
        ], 1)                                        # (NBL, 97, 192)
        w2bk = np.concatenate([-w2i, W2c], 2)        # (NBL, 96, 192)

        im = {
            "xh": np.ascontiguousarray(
                x[b, cs].transpose(1, 0, 2)).astype(BF16),   # [H, C, W]
            "w1k": w1kk.astype(BF16),
            "badd": baddk.astype(np.float32),
            "w2a": w2ak.astype(BF16),
            "w2b": w2bk.astype(BF16),
        }
        im.update(consts)
        in_maps.append(im)

    import os as _os
    trace = bool(int(_os.environ.get("AFNO_TRACE", "0")))
    res = run_bass_kernel_spmd(
        nc, in_maps, core_ids=list(range(N_CORES)), trace=trace
    )
    global LAST_RESULTS
    LAST_RESULTS = res

    out = np.empty((B_FULL, DIM, H, W), dtype=np.float32)
    for core in range(N_CORES):
        b = core // 2
        n0 = (core % 2) * NBL
        cs = slice(n0 * BS, n0 * BS + C)
        out[b, cs] = res.results[core]["outs"].transpose(1, 0, 2)
    return out
